# revision 9
# baseline (speedup 1.0000x reference)
"""Trainium2 Bass kernel for nn_Attention_42417097015520.

Full-input contract: kernel(**inputs) takes the unsharded inputs
(x [4,2048,768], W_qkv [768,2304], W_proj [768,768], b_proj [768]) and
returns the full [4,2048,768] output.

Sharding (8 cores): core c handles batch b=c//2 and heads
h in [(c%2)*6, (c%2)*6+6) (tensor parallel over heads x data parallel
over batch). Each core computes its 6 heads' attention plus the partial
output projection against its 384-row slice of W_proj; the host sums the
two partials per batch and adds b_proj.

Device-side layout/algorithm (per core, identical SPMD program):
  - inputs: xT = x[b].T [768,2048], wqkv = W_qkv column slice [768,1152]
    (q|k|v blocks of 384), wproj row slice [384,768].
  - QK^T projection -> qkT_sb [128,6,2048] (planes 0-2: Q^T, 3-5: K^T;
    head h lives on partitions (h%2)*64.. of plane h//2).
  - V projection -> v_sb [128,16,6*65]: per head 64 V columns plus a ones
    column (the ones column makes the P@V matmul also emit the softmax
    denominators as PSUM row 64).
  - Attention per head: S^T chunk [128m,512n] = K^T_chunk.T @ Q^T via PE
    (f32r), exp via ScalarE (scale=1/8 folded in), O^T = V.T @ P^T
    accumulated over m-chunks, then normalize by broadcast reciprocal of
    the denominator row.
  - Output projection from the O^T layout (heads on partitions), partial
    result [2048,768] DMA'd out.
"""

import sys
import types
import contextlib
import ctypes
from contextlib import ExitStack

import numpy as np

import concourse.bass as bass
import concourse.mybir as mybir
import concourse.tile as tile
from concourse.bass_utils import run_bass_kernel_spmd

B, N, D, H, HD = 4, 2048, 768, 12, 64
HPC = H // 2          # heads per core = 6
NCORES = 8
SCALE = HD ** -0.5    # 0.125
F32 = mybir.dt.float32
F32R = mybir.dt.float32r
P = 128
VW = HD + 1           # V columns per head incl. ones column = 65


# ---------------------------------------------------------------------------
# Workaround: this container's walrus accepts at most ONE sem wait per
# instruction. Hoist extra waits onto same-engine NoOps inserted before.
# ---------------------------------------------------------------------------
_wsplit_ctr = [0]


def _split_waits(nc, cap: int = 1) -> int:
    n_split = 0
    for f in nc.m.functions:
        for bb in f.blocks:
            insts = list(bb.instructions)
            out = []
            for ins in insts:
                si = ins.sync_info
                if si is not None and si.on_wait and len(si.on_wait) > cap:
                    waits = list(si.on_wait)
                    for i in range(0, len(waits) - cap, cap):
                        _wsplit_ctr[0] += 1
                        out.append(
                            mybir.InstNoOp(
                                name=f"I-wsplit-{_wsplit_ctr[0]}",
                                engine=ins.engine,
                                ins=[],
                                outs=[],
                                sync_info=mybir.SyncInfo(
                                    on_wait=waits[i : i + cap], on_update=[]
                                ),
                            )
                        )
                    si.on_wait = waits[len(waits) - cap :]
                    n_split += 1
                out.append(ins)
            if len(out) != len(insts):
                bb.instructions[:] = out
    return n_split


# ---------------------------------------------------------------------------
# NTFF profiling shim (the image's antenv lacks axon_hooks); only needed
# when trace=True is requested.
# ---------------------------------------------------------------------------
_HOOK = [None]


def _install_ntff_shim():
    if "antenv.axon_hooks" in sys.modules:
        return
    mod = types.ModuleType("antenv.axon_hooks")
    mod.set_axon_ntff_profile_hook = lambda h: _HOOK.__setitem__(0, h)
    mod.get_axon_ntff_profile_hook = lambda: _HOOK[0]
    sys.modules["antenv.axon_hooks"] = mod
    try:
        import antenv

        antenv.axon_hooks = mod
    except ImportError:
        pass

    try:
        lib = ctypes.CDLL("/opt/axon/libaxon_pjrt.so")
    except OSError:
        return
    if not hasattr(lib, "axon_start_nrt_profile"):
        return
    lib.axon_start_nrt_profile.argtypes = [
        ctypes.POINTER(ctypes.c_int64),
        ctypes.c_size_t,
    ]
    lib.axon_start_nrt_profile.restype = ctypes.c_int64
    lib.axon_stop_nrt_profile.argtypes = [ctypes.c_char_p]
    lib.axon_stop_nrt_profile.restype = ctypes.c_int64

    @contextlib.contextmanager
    def _hook(output_dir, device_ids):
        import jax

        jax.devices()
        if device_ids:
            ids = (ctypes.c_int64 * len(device_ids))(*device_ids)
            rc = lib.axon_start_nrt_profile(ids, len(device_ids))
        else:
            rc = lib.axon_start_nrt_profile(None, 0)
        if rc != 0:
            raise RuntimeError(f"axon_start_nrt_profile rc={rc}")
        try:
            yield
        finally:
            n = lib.axon_stop_nrt_profile(str(output_dir).encode())
            if n < 0:
                raise RuntimeError(f"axon_stop_nrt_profile rc={n}")

    _HOOK[0] = _hook

    import concourse.bass_utils as bu

    bu.upload_artifacts = lambda tmpdir: str(tmpdir)


# ---------------------------------------------------------------------------
# Device program
# ---------------------------------------------------------------------------
def _build_nc():
    nc = bass.Bass()
    xT = nc.declare_dram_parameter("xT", [D, N], F32R, isOutput=False).ap()
    wqkv = nc.declare_dram_parameter("wqkv", [D, 3 * HPC * HD], F32R, isOutput=False).ap()
    wproj = nc.declare_dram_parameter("wproj", [HPC * HD, D], F32R, isOutput=False).ap()
    ones_in = nc.declare_dram_parameter(
        "ones", [(N // P) * (H // 2)], F32R, isOutput=False
    ).ap()
    out = nc.declare_dram_parameter("out", [N, D], F32, isOutput=True).ap()

    DO = D // P          # 6 d-chunks of 128
    NB = N // 512        # 4 n-blocks of 512
    MC = N // P          # 16 m-chunks of 128
    QKC = (2 * HPC * HD) // P   # 6 column blocks of q|k

    with tile.TileContext(nc) as tc, ExitStack() as ctx:
        persist = ctx.enter_context(tc.tile_pool(name="persist", bufs=1))
        outcp = ctx.enter_context(tc.tile_pool(name="outcp", bufs=3))
        small = ctx.enter_context(tc.tile_pool(name="small", bufs=4))
        dramp = ctx.enter_context(tc.tile_pool(name="dramp", bufs=2, space="DRAM"))
        psum_mm = ctx.enter_context(tc.tile_pool(name="psum_mm", bufs=2, space="PSUM"))
        psum_s = ctx.enter_context(tc.tile_pool(name="psum_s", bufs=2, space="PSUM"))
        psum_o = ctx.enter_context(tc.tile_pool(name="psum_o", bufs=2, space="PSUM"))

        qkT_sb = persist.tile([P, 2 * HPC * HD // P, N], F32R)   # [128, 6, 2048]
        v_sb = persist.tile([P, MC, HPC * VW], F32R)             # [128, 16, 390]
        oT_sb = persist.tile([P, HPC * HD // P, N], F32R)        # [128, 3, 2048]
        wp_sb = persist.tile([P, HPC * HD // P, D], F32R)        # [128, 3, 768]

        for p3 in range(HPC * HD // P):
            nc.sync.dma_start(out=wp_sb[:, p3, :], in_=wproj[p3 * P:(p3 + 1) * P, :])
        # ones columns for the softmax-denominator rows, DMA'd from a tiny
        # host-provided tensor with a partition-step-0 broadcast read
        nc.sync.dma_start(
            out=v_sb.rearrange("p m (h c) -> p m h c", c=VW)[:, :, :, HD],
            in_=bass.AP(
                tensor=ones_in.tensor,
                offset=ones_in.offset,
                ap=[[0, P], [1, MC * HPC]],
            ),
        )

        with ExitStack() as early_ctx:
            early = early_ctx.enter_context(tc.tile_pool(name="early", bufs=1))
            xT_sb = early.tile([P, DO, N], F32R)                  # [128, 6, 2048]
            wqkv_sb = early.tile([P, DO, 3 * HPC * HD], F32R)     # [128, 6, 1152]
            for o in range(DO):
                nc.sync.dma_start(out=xT_sb[:, o, :], in_=xT[o * P:(o + 1) * P, :])
                nc.sync.dma_start(
                    out=wqkv_sb[:, o, :], in_=wqkv[o * P:(o + 1) * P, :]
                )

            # ---- Q^T / K^T projection: qkT = (wqkv_qk).T @ x.T ----
            for cb in range(QKC):
                for nb in range(NB):
                    ps = psum_mm.tile([P, 512], F32)
                    for o in range(DO):
                        nc.tensor.matmul(
                            ps[:, :],
                            lhsT=wqkv_sb[:, o, cb * P:(cb + 1) * P],
                            rhs=xT_sb[:, o, nb * 512:(nb + 1) * 512],
                            start=(o == 0),
                            stop=(o == DO - 1),
                        )
                    nc.vector.tensor_copy(
                        qkT_sb[:, cb, nb * 512:(nb + 1) * 512], ps[:, :]
                    )

            # ---- V projection: v = x @ wqkv_v (row chunks) ----
            vcol0 = 2 * HPC * HD
            for mc in range(MC):
                ps = psum_mm.tile([P, 512], F32)
                for o in range(DO):
                    nc.tensor.matmul(
                        ps[:, : HPC * HD],
                        lhsT=xT_sb[:, o, mc * P:(mc + 1) * P],
                        rhs=wqkv_sb[:, o, vcol0: vcol0 + HPC * HD],
                        start=(o == 0),
                        stop=(o == DO - 1),
                    )
                nc.vector.tensor_copy(
                    v_sb.rearrange("p m (h c) -> p m h c", c=VW)[:, mc, :, 0:HD],
                    ps[:, : HPC * HD].rearrange("p (h c) -> p h c", c=HD),
                )

        # ---- attention (pT pool reuses the freed early space) ----
        with ExitStack() as attn_ctx:
            ptp = attn_ctx.enter_context(tc.tile_pool(name="ptp", bufs=2))
            for h in range(HPC):
                kb = (h % 2) * HD
                qpl = h // 2
                kpl = HPC * HD // P + h // 2   # 3 + h//2
                for nb in range(NB):
                    pT = ptp.tile([P, MC, 512], F32R)
                    for g in range(MC // 2):
                        ps = psum_s.tile([P, 1024], F32)
                        for j in range(2):
                            mc = 2 * g + j
                            nc.tensor.matmul(
                                ps[:, j * 512:(j + 1) * 512],
                                lhsT=qkT_sb[kb:kb + HD, kpl, mc * P:(mc + 1) * P],
                                rhs=qkT_sb[kb:kb + HD, qpl, nb * 512:(nb + 1) * 512],
                                start=True,
                                stop=True,
                            )
                        nc.scalar.activation(
                            pT[:, 2 * g:2 * g + 2, :].rearrange("p a b -> p (a b)"),
                            ps[:, :],
                            mybir.ActivationFunctionType.Exp,
                            scale=SCALE,
                        )
                    po = psum_o.tile([P, 512], F32)
                    for mc in range(MC):
                        nc.tensor.matmul(
                            po[:VW, :],
                            lhsT=v_sb[:, mc, h * VW:(h + 1) * VW],
                            rhs=pT[:, mc, :],
                            start=(mc == 0),
                            stop=(mc == MC - 1),
                        )
                    recip = small.tile([1, 512], F32)
                    nc.vector.reciprocal(recip[:, :], po[HD:VW, :])
                    # broadcast the reciprocal row to 64 partitions via a
                    # DRAM bounce (partition-step-0 read)
                    dscratch = dramp.tile([1, 512], F32)
                    nc.sync.dma_start(out=dscratch[:, :], in_=recip[:, :])
                    rbc = small.tile([HD, 512], F32)
                    nc.sync.dma_start(
                        out=rbc[:, :],
                        in_=bass.AP(
                            tensor=dscratch.tensor,
                            offset=dscratch.offset,
                            ap=[[0, HD], [1, 512]],
                        ),
                    )
                    nc.vector.tensor_mul(
                        oT_sb[kb:kb + HD, qpl, nb * 512:(nb + 1) * 512],
                        po[0:HD, :],
                        rbc[:, :],
                    )

            # ---- output projection: out = oT.T @ wproj ----
            PH = HPC * HD // P  # 3
            for mc in range(MC):
                for half in range(2):
                    ps = psum_mm.tile([P, 512], F32)
                    for p3 in range(PH):
                        nc.tensor.matmul(
                            ps[:, : D // 2],
                            lhsT=oT_sb[:, p3, mc * P:(mc + 1) * P],
                            rhs=wp_sb[:, p3, half * (D // 2):(half + 1) * (D // 2)],
                            start=(p3 == 0),
                            stop=(p3 == PH - 1),
                        )
                    oc = outcp.tile([P, D // 2], F32)
                    nc.vector.tensor_copy(oc[:, :], ps[:, : D // 2])
                    nc.sync.dma_start(
                        out=out[mc * P:(mc + 1) * P, half * (D // 2):(half + 1) * (D // 2)],
                        in_=oc[:, :],
                    )

    _split_waits(nc)
    return nc


_NC_CACHE = [None]


def _get_nc():
    if _NC_CACHE[0] is None:
        _NC_CACHE[0] = _build_nc()
    return _NC_CACHE[0]


def _to_f32r(a):
    """Round fp32 to the fp32r format (11-bit mantissa, low 12 bits zero),
    round-to-nearest-even. The device consumes these tensors as f32r."""
    u = np.ascontiguousarray(a, dtype=np.float32).view(np.uint32).copy()
    lsb = (u >> np.uint32(12)) & np.uint32(1)
    u += np.uint32(0x7FF) + lsb
    u &= np.uint32(0xFFFFF000)
    return u.view(np.float32)


def _make_in_maps(x, W_qkv, W_proj):
    in_maps = []
    for c in range(NCORES):
        b = c // 2
        h0 = (c % 2) * HPC
        qcols = W_qkv[:, h0 * HD:(h0 + HPC) * HD]
        kcols = W_qkv[:, D + h0 * HD: D + (h0 + HPC) * HD]
        vcols = W_qkv[:, 2 * D + h0 * HD: 2 * D + (h0 + HPC) * HD]
        in_maps.append(
            {
                "xT": _to_f32r(x[b].T),
                "wqkv": _to_f32r(np.concatenate([qcols, kcols, vcols], axis=1)),
                "wproj": _to_f32r(W_proj[h0 * HD:(h0 + HPC) * HD, :]),
                "ones": np.ones((N // P) * (H // 2), dtype=np.float32),
            }
        )
    return in_maps


def _run(inputs, trace=False):
    x = np.asarray(inputs["x"], dtype=np.float32)
    W_qkv = np.asarray(inputs["W_qkv"], dtype=np.float32)
    W_proj = np.asarray(inputs["W_proj"], dtype=np.float32)
    b_proj = np.asarray(inputs["b_proj"], dtype=np.float32)

    if trace:
        _install_ntff_shim()
    nc = _get_nc()
    res = run_bass_kernel_spmd(
        nc, _make_in_maps(x, W_qkv, W_proj), core_ids=list(range(NCORES)),
        trace=trace,
    )
    parts = res.results
    out = np.empty((B, N, D), dtype=np.float32)
    for b in range(B):
        out[b] = parts[2 * b]["out"] + parts[2 * b + 1]["out"] + b_proj
    return out, res


def kernel(**inputs) -> np.ndarray:
    out, _ = _run(inputs, trace=False)
    return out


def run_traced(inputs):
    return _run(inputs, trace=True)


# revision 13
# speedup vs baseline: 1.0427x; 1.0427x over previous
"""Trainium2 Bass kernel for nn_Attention_42417097015520.

Full-input contract: kernel(**inputs) takes the unsharded inputs
(x [4,2048,768], W_qkv [768,2304], W_proj [768,768], b_proj [768]) and
returns the full [4,2048,768] output.

Sharding (8 cores): core c handles batch b=c//2 and heads
h in [(c%2)*6, (c%2)*6+6) (tensor parallel over heads x data parallel
over batch). Each core computes its 6 heads' attention plus the partial
output projection against its 384-row slice of W_proj; the host sums the
two partials per batch and adds b_proj.

Device-side layout/algorithm (per core, identical SPMD program):
  - inputs: xT = x[b].T [768,2048], wqkv = W_qkv column slice [768,1152]
    (q|k|v blocks of 384), wproj row slice [384,768].
  - QK^T projection -> qkT_sb [128,6,2048] (planes 0-2: Q^T, 3-5: K^T;
    head h lives on partitions (h%2)*64.. of plane h//2).
  - V projection -> v_sb [128,16,6*65]: per head 64 V columns plus a ones
    column (the ones column makes the P@V matmul also emit the softmax
    denominators as PSUM row 64).
  - Attention per head: S^T chunk [128m,512n] = K^T_chunk.T @ Q^T via PE
    (f32r), exp via ScalarE (scale=1/8 folded in), O^T = V.T @ P^T
    accumulated over m-chunks, then normalize by broadcast reciprocal of
    the denominator row.
  - Output projection from the O^T layout (heads on partitions), partial
    result [2048,768] DMA'd out.
"""

import sys
import types
import contextlib
import ctypes
from contextlib import ExitStack

import numpy as np

import concourse.bass as bass
import concourse.mybir as mybir
import concourse.tile as tile
from concourse.bass_utils import run_bass_kernel_spmd

B, N, D, H, HD = 4, 2048, 768, 12, 64
HPC = H // 2          # heads per core = 6
NCORES = 8
SCALE = HD ** -0.5    # 0.125
F32 = mybir.dt.float32
F32R = mybir.dt.float32r
BF16 = mybir.dt.bfloat16
P = 128
VW = HD + 1           # V columns per head incl. ones column = 65


# ---------------------------------------------------------------------------
# Workaround: this container's walrus accepts at most ONE sem wait per
# instruction. Hoist extra waits onto same-engine NoOps inserted before.
# ---------------------------------------------------------------------------
_wsplit_ctr = [0]


def _split_waits(nc, cap: int = 1) -> int:
    n_split = 0
    for f in nc.m.functions:
        for bb in f.blocks:
            insts = list(bb.instructions)
            out = []
            for ins in insts:
                si = ins.sync_info
                if si is not None and si.on_wait and len(si.on_wait) > cap:
                    waits = list(si.on_wait)
                    for i in range(0, len(waits) - cap, cap):
                        _wsplit_ctr[0] += 1
                        out.append(
                            mybir.InstNoOp(
                                name=f"I-wsplit-{_wsplit_ctr[0]}",
                                engine=ins.engine,
                                ins=[],
                                outs=[],
                                sync_info=mybir.SyncInfo(
                                    on_wait=waits[i : i + cap], on_update=[]
                                ),
                            )
                        )
                    si.on_wait = waits[len(waits) - cap :]
                    n_split += 1
                out.append(ins)
            if len(out) != len(insts):
                bb.instructions[:] = out
    return n_split


# ---------------------------------------------------------------------------
# NTFF profiling shim (the image's antenv lacks axon_hooks); only needed
# when trace=True is requested.
# ---------------------------------------------------------------------------
_HOOK = [None]


def _install_ntff_shim():
    if "antenv.axon_hooks" in sys.modules:
        return
    mod = types.ModuleType("antenv.axon_hooks")
    mod.set_axon_ntff_profile_hook = lambda h: _HOOK.__setitem__(0, h)
    mod.get_axon_ntff_profile_hook = lambda: _HOOK[0]
    sys.modules["antenv.axon_hooks"] = mod
    try:
        import antenv

        antenv.axon_hooks = mod
    except ImportError:
        pass

    try:
        lib = ctypes.CDLL("/opt/axon/libaxon_pjrt.so")
    except OSError:
        return
    if not hasattr(lib, "axon_start_nrt_profile"):
        return
    lib.axon_start_nrt_profile.argtypes = [
        ctypes.POINTER(ctypes.c_int64),
        ctypes.c_size_t,
    ]
    lib.axon_start_nrt_profile.restype = ctypes.c_int64
    lib.axon_stop_nrt_profile.argtypes = [ctypes.c_char_p]
    lib.axon_stop_nrt_profile.restype = ctypes.c_int64

    @contextlib.contextmanager
    def _hook(output_dir, device_ids):
        import jax

        jax.devices()
        if device_ids:
            ids = (ctypes.c_int64 * len(device_ids))(*device_ids)
            rc = lib.axon_start_nrt_profile(ids, len(device_ids))
        else:
            rc = lib.axon_start_nrt_profile(None, 0)
        if rc != 0:
            raise RuntimeError(f"axon_start_nrt_profile rc={rc}")
        try:
            yield
        finally:
            n = lib.axon_stop_nrt_profile(str(output_dir).encode())
            if n < 0:
                raise RuntimeError(f"axon_stop_nrt_profile rc={n}")

    _HOOK[0] = _hook

    import concourse.bass_utils as bu

    bu.upload_artifacts = lambda tmpdir: str(tmpdir)


# ---------------------------------------------------------------------------
# Device program
# ---------------------------------------------------------------------------
def _build_nc():
    nc = bass.Bass()
    xT = nc.declare_dram_parameter("xT", [D, N], BF16, isOutput=False).ap()
    wqkv = nc.declare_dram_parameter("wqkv", [D, 3 * HPC * HD], BF16, isOutput=False).ap()
    wproj = nc.declare_dram_parameter("wproj", [HPC * HD, D], BF16, isOutput=False).ap()
    ones_in = nc.declare_dram_parameter(
        "ones", [(N // P) * (H // 2)], BF16, isOutput=False
    ).ap()
    out = nc.declare_dram_parameter("out", [N, D], F32, isOutput=True).ap()

    DO = D // P          # 6 d-chunks of 128
    NB = N // 512        # 4 n-blocks of 512
    MC = N // P          # 16 m-chunks of 128
    QKC = (2 * HPC * HD) // P   # 6 column blocks of q|k

    with tile.TileContext(nc) as tc, ExitStack() as ctx:
        persist = ctx.enter_context(tc.tile_pool(name="persist", bufs=1))
        outcp = ctx.enter_context(tc.tile_pool(name="outcp", bufs=3))
        small = ctx.enter_context(tc.tile_pool(name="small", bufs=4))
        dramp = ctx.enter_context(tc.tile_pool(name="dramp", bufs=2, space="DRAM"))
        psum_mm = ctx.enter_context(tc.tile_pool(name="psum_mm", bufs=2, space="PSUM"))
        psum_s = ctx.enter_context(tc.tile_pool(name="psum_s", bufs=2, space="PSUM"))
        psum_o = ctx.enter_context(tc.tile_pool(name="psum_o", bufs=2, space="PSUM"))

        qkT_sb = persist.tile([P, 2 * HPC * HD // P, N], BF16)   # [128, 6, 2048]
        v_sb = persist.tile([P, MC, HPC * VW], BF16)             # [128, 16, 390]
        oT_sb = persist.tile([P, HPC * HD // P, N], BF16)        # [128, 3, 2048]
        wp_sb = persist.tile([P, HPC * HD // P, D], BF16)        # [128, 3, 768]

        for p3 in range(HPC * HD // P):
            nc.sync.dma_start(out=wp_sb[:, p3, :], in_=wproj[p3 * P:(p3 + 1) * P, :])
        # ones columns for the softmax-denominator rows, DMA'd from a tiny
        # host-provided tensor with a partition-step-0 broadcast read
        nc.sync.dma_start(
            out=v_sb.rearrange("p m (h c) -> p m h c", c=VW)[:, :, :, HD],
            in_=bass.AP(
                tensor=ones_in.tensor,
                offset=ones_in.offset,
                ap=[[0, P], [1, MC * HPC]],
            ),
        )

        with ExitStack() as early_ctx:
            early = early_ctx.enter_context(tc.tile_pool(name="early", bufs=1))
            xT_sb = early.tile([P, DO, N], BF16)                  # [128, 6, 2048]
            wqkv_sb = early.tile([P, DO, 3 * HPC * HD], BF16)     # [128, 6, 1152]
            for o in range(DO):
                nc.sync.dma_start(out=xT_sb[:, o, :], in_=xT[o * P:(o + 1) * P, :])
                nc.sync.dma_start(
                    out=wqkv_sb[:, o, :], in_=wqkv[o * P:(o + 1) * P, :]
                )

            # ---- Q^T / K^T projection: qkT = (wqkv_qk).T @ x.T ----
            for cb in range(QKC):
                for nb in range(NB):
                    ps = psum_mm.tile([P, 512], F32)
                    for o in range(DO):
                        nc.tensor.matmul(
                            ps[:, :],
                            lhsT=wqkv_sb[:, o, cb * P:(cb + 1) * P],
                            rhs=xT_sb[:, o, nb * 512:(nb + 1) * 512],
                            start=(o == 0),
                            stop=(o == DO - 1),
                        )
                    nc.vector.tensor_copy(
                        qkT_sb[:, cb, nb * 512:(nb + 1) * 512], ps[:, :]
                    )

            # ---- V projection: v = x @ wqkv_v (row chunks) ----
            vcol0 = 2 * HPC * HD
            for mc in range(MC):
                ps = psum_mm.tile([P, 512], F32)
                for o in range(DO):
                    nc.tensor.matmul(
                        ps[:, : HPC * HD],
                        lhsT=xT_sb[:, o, mc * P:(mc + 1) * P],
                        rhs=wqkv_sb[:, o, vcol0: vcol0 + HPC * HD],
                        start=(o == 0),
                        stop=(o == DO - 1),
                    )
                nc.vector.tensor_copy(
                    v_sb.rearrange("p m (h c) -> p m h c", c=VW)[:, mc, :, 0:HD],
                    ps[:, : HPC * HD].rearrange("p (h c) -> p h c", c=HD),
                )

        # ---- attention (pT pool reuses the freed early space) ----
        with ExitStack() as attn_ctx:
            ptp = attn_ctx.enter_context(tc.tile_pool(name="ptp", bufs=2))
            for h in range(HPC):
                kb = (h % 2) * HD
                qpl = h // 2
                kpl = HPC * HD // P + h // 2   # 3 + h//2
                for nb in range(NB):
                    pT = ptp.tile([P, MC, 512], BF16)
                    po = psum_o.tile([P, 512], F32)
                    # interleave S-matmuls/exp with the O accumulation so the
                    # PE stream stays dense while ScalarE works through exps
                    for g in range(MC // 2):
                        ps = psum_s.tile([P, 1024], F32)
                        for j in range(2):
                            mc = 2 * g + j
                            nc.tensor.matmul(
                                ps[:, j * 512:(j + 1) * 512],
                                lhsT=qkT_sb[kb:kb + HD, kpl, mc * P:(mc + 1) * P],
                                rhs=qkT_sb[kb:kb + HD, qpl, nb * 512:(nb + 1) * 512],
                                start=True,
                                stop=True,
                            )
                        nc.scalar.activation(
                            pT[:, 2 * g:2 * g + 2, :].rearrange("p a b -> p (a b)"),
                            ps[:, :],
                            mybir.ActivationFunctionType.Exp,
                            scale=SCALE,
                        )
                        if g >= 1:
                            for mc in (2 * g - 2, 2 * g - 1):
                                nc.tensor.matmul(
                                    po[:VW, :],
                                    lhsT=v_sb[:, mc, h * VW:(h + 1) * VW],
                                    rhs=pT[:, mc, :],
                                    start=(mc == 0),
                                    stop=False,
                                )
                    for mc in (MC - 2, MC - 1):
                        nc.tensor.matmul(
                            po[:VW, :],
                            lhsT=v_sb[:, mc, h * VW:(h + 1) * VW],
                            rhs=pT[:, mc, :],
                            start=False,
                            stop=(mc == MC - 1),
                        )
                    # move the accumulated [65,512] out of PSUM promptly,
                    # then normalize: broadcast the denominator row via a
                    # DRAM bounce and apply a fast reciprocal on 64 lanes
                    oTu = small.tile([VW, 512], F32)
                    nc.vector.tensor_copy(oTu[:, :], po[:VW, :])
                    # denominators: 512 values on ONE partition. Spread them
                    # over 64 partitions via a DRAM bounce so the exact DVE
                    # reciprocal runs on 64 lanes, then broadcast back.
                    dsums = dramp.tile([1, 512], F32)
                    nc.gpsimd.dma_start(out=dsums[:, :], in_=oTu[HD:VW, :])
                    spread = small.tile([HD, 8], F32)
                    nc.gpsimd.dma_start(
                        out=spread[:, :],
                        in_=bass.AP(
                            tensor=dsums.tensor,
                            offset=dsums.offset,
                            ap=[[8, HD], [1, 8]],
                        ),
                    )
                    rspread = small.tile([HD, 8], F32)
                    nc.vector.reciprocal(rspread[:, :], spread[:, :])
                    drcp = dramp.tile([1, 512], F32)
                    nc.gpsimd.dma_start(
                        out=bass.AP(
                            tensor=drcp.tensor,
                            offset=drcp.offset,
                            ap=[[8, HD], [1, 8]],
                        ),
                        in_=rspread[:, :],
                    )
                    rcp = small.tile([HD, 512], F32)
                    nc.gpsimd.dma_start(
                        out=rcp[:, :],
                        in_=bass.AP(
                            tensor=drcp.tensor,
                            offset=drcp.offset,
                            ap=[[0, HD], [1, 512]],
                        ),
                    )
                    nc.vector.tensor_mul(
                        oT_sb[kb:kb + HD, qpl, nb * 512:(nb + 1) * 512],
                        oTu[0:HD, :],
                        rcp[:, :],
                    )

            # ---- output projection: out = oT.T @ wproj ----
            PH = HPC * HD // P  # 3
            for mc in range(MC):
                for half in range(2):
                    ps = psum_mm.tile([P, 512], F32)
                    for p3 in range(PH):
                        nc.tensor.matmul(
                            ps[:, : D // 2],
                            lhsT=oT_sb[:, p3, mc * P:(mc + 1) * P],
                            rhs=wp_sb[:, p3, half * (D // 2):(half + 1) * (D // 2)],
                            start=(p3 == 0),
                            stop=(p3 == PH - 1),
                        )
                    oc = outcp.tile([P, D // 2], F32)
                    nc.vector.tensor_copy(oc[:, :], ps[:, : D // 2])
                    nc.sync.dma_start(
                        out=out[mc * P:(mc + 1) * P, half * (D // 2):(half + 1) * (D // 2)],
                        in_=oc[:, :],
                    )

    _split_waits(nc)
    return nc


_NC_CACHE = [None]


def _get_nc():
    if _NC_CACHE[0] is None:
        _NC_CACHE[0] = _build_nc()
    return _NC_CACHE[0]


def _make_in_maps(x, W_qkv, W_proj):
    import ml_dtypes

    bf16 = ml_dtypes.bfloat16
    in_maps = []
    for c in range(NCORES):
        b = c // 2
        h0 = (c % 2) * HPC
        qcols = W_qkv[:, h0 * HD:(h0 + HPC) * HD]
        kcols = W_qkv[:, D + h0 * HD: D + (h0 + HPC) * HD]
        vcols = W_qkv[:, 2 * D + h0 * HD: 2 * D + (h0 + HPC) * HD]
        in_maps.append(
            {
                "xT": np.ascontiguousarray(x[b].T).astype(bf16),
                "wqkv": np.concatenate([qcols, kcols, vcols], axis=1).astype(bf16),
                "wproj": np.ascontiguousarray(
                    W_proj[h0 * HD:(h0 + HPC) * HD, :]
                ).astype(bf16),
                "ones": np.ones((N // P) * (H // 2), dtype=bf16),
            }
        )
    return in_maps


def _run(inputs, trace=False):
    x = np.asarray(inputs["x"], dtype=np.float32)
    W_qkv = np.asarray(inputs["W_qkv"], dtype=np.float32)
    W_proj = np.asarray(inputs["W_proj"], dtype=np.float32)
    b_proj = np.asarray(inputs["b_proj"], dtype=np.float32)

    if trace:
        _install_ntff_shim()
    nc = _get_nc()
    res = run_bass_kernel_spmd(
        nc, _make_in_maps(x, W_qkv, W_proj), core_ids=list(range(NCORES)),
        trace=trace,
    )
    parts = res.results
    out = np.empty((B, N, D), dtype=np.float32)
    for b in range(B):
        out[b] = parts[2 * b]["out"] + parts[2 * b + 1]["out"] + b_proj
    return out, res


def kernel(**inputs) -> np.ndarray:
    out, _ = _run(inputs, trace=False)
    return out


def run_traced(inputs):
    return _run(inputs, trace=True)


# revision 15
# speedup vs baseline: 1.2459x; 1.1948x over previous
"""Trainium2 Bass kernel for nn_Attention_42417097015520.

Full-input contract: kernel(**inputs) takes the unsharded inputs
(x [4,2048,768], W_qkv [768,2304], W_proj [768,768], b_proj [768]) and
returns the full [4,2048,768] output.

Sharding (8 cores): core c handles batch b=c//2 and heads
h in [(c%2)*6, (c%2)*6+6) (tensor parallel over heads x data parallel
over batch). Each core computes its 6 heads' attention plus the partial
output projection against its 384-row slice of W_proj; the host sums the
two partials per batch and adds b_proj.

Device-side layout/algorithm (per core, identical SPMD program):
  - inputs: xT = x[b].T [768,2048], wqkv = W_qkv column slice [768,1152]
    (q|k|v blocks of 384), wproj row slice [384,768].
  - QK^T projection -> qkT_sb [128,6,2048] (planes 0-2: Q^T, 3-5: K^T;
    head h lives on partitions (h%2)*64.. of plane h//2).
  - V projection -> v_sb [128,16,6*65]: per head 64 V columns plus a ones
    column (the ones column makes the P@V matmul also emit the softmax
    denominators as PSUM row 64).
  - Attention per head: S^T chunk [128m,512n] = K^T_chunk.T @ Q^T via PE
    (f32r), exp via ScalarE (scale=1/8 folded in), O^T = V.T @ P^T
    accumulated over m-chunks, then normalize by broadcast reciprocal of
    the denominator row.
  - Output projection from the O^T layout (heads on partitions), partial
    result [2048,768] DMA'd out.
"""

import sys
import types
import contextlib
import ctypes
from contextlib import ExitStack

import numpy as np

import concourse.bass as bass
import concourse.mybir as mybir
import concourse.tile as tile
from concourse.bass_utils import run_bass_kernel_spmd

B, N, D, H, HD = 4, 2048, 768, 12, 64
HPC = H // 2          # heads per core = 6
NCORES = 8
SCALE = HD ** -0.5    # 0.125
F32 = mybir.dt.float32
F32R = mybir.dt.float32r
BF16 = mybir.dt.bfloat16
P = 128
VW = HD + 1           # V columns per head incl. ones column = 65


# ---------------------------------------------------------------------------
# Workaround: this container's walrus accepts at most ONE sem wait per
# instruction. Hoist extra waits onto same-engine NoOps inserted before.
# ---------------------------------------------------------------------------
_wsplit_ctr = [0]


def _split_waits(nc, cap: int = 1) -> int:
    n_split = 0
    for f in nc.m.functions:
        for bb in f.blocks:
            insts = list(bb.instructions)
            out = []
            for ins in insts:
                si = ins.sync_info
                if si is not None and si.on_wait and len(si.on_wait) > cap:
                    waits = list(si.on_wait)
                    for i in range(0, len(waits) - cap, cap):
                        _wsplit_ctr[0] += 1
                        out.append(
                            mybir.InstNoOp(
                                name=f"I-wsplit-{_wsplit_ctr[0]}",
                                engine=ins.engine,
                                ins=[],
                                outs=[],
                                sync_info=mybir.SyncInfo(
                                    on_wait=waits[i : i + cap], on_update=[]
                                ),
                            )
                        )
                    si.on_wait = waits[len(waits) - cap :]
                    n_split += 1
                out.append(ins)
            if len(out) != len(insts):
                bb.instructions[:] = out
    return n_split


# ---------------------------------------------------------------------------
# NTFF profiling shim (the image's antenv lacks axon_hooks); only needed
# when trace=True is requested.
# ---------------------------------------------------------------------------
_HOOK = [None]


def _install_ntff_shim():
    if "antenv.axon_hooks" in sys.modules:
        return
    mod = types.ModuleType("antenv.axon_hooks")
    mod.set_axon_ntff_profile_hook = lambda h: _HOOK.__setitem__(0, h)
    mod.get_axon_ntff_profile_hook = lambda: _HOOK[0]
    sys.modules["antenv.axon_hooks"] = mod
    try:
        import antenv

        antenv.axon_hooks = mod
    except ImportError:
        pass

    try:
        lib = ctypes.CDLL("/opt/axon/libaxon_pjrt.so")
    except OSError:
        return
    if not hasattr(lib, "axon_start_nrt_profile"):
        return
    lib.axon_start_nrt_profile.argtypes = [
        ctypes.POINTER(ctypes.c_int64),
        ctypes.c_size_t,
    ]
    lib.axon_start_nrt_profile.restype = ctypes.c_int64
    lib.axon_stop_nrt_profile.argtypes = [ctypes.c_char_p]
    lib.axon_stop_nrt_profile.restype = ctypes.c_int64

    @contextlib.contextmanager
    def _hook(output_dir, device_ids):
        import jax

        jax.devices()
        if device_ids:
            ids = (ctypes.c_int64 * len(device_ids))(*device_ids)
            rc = lib.axon_start_nrt_profile(ids, len(device_ids))
        else:
            rc = lib.axon_start_nrt_profile(None, 0)
        if rc != 0:
            raise RuntimeError(f"axon_start_nrt_profile rc={rc}")
        try:
            yield
        finally:
            n = lib.axon_stop_nrt_profile(str(output_dir).encode())
            if n < 0:
                raise RuntimeError(f"axon_stop_nrt_profile rc={n}")

    _HOOK[0] = _hook

    import concourse.bass_utils as bu

    bu.upload_artifacts = lambda tmpdir: str(tmpdir)


# ---------------------------------------------------------------------------
# Device program
# ---------------------------------------------------------------------------
def _build_nc():
    nc = bass.Bass()
    xT = nc.declare_dram_parameter("xT", [D, N], BF16, isOutput=False).ap()
    wqkv = nc.declare_dram_parameter("wqkv", [D, 3 * HPC * HD], BF16, isOutput=False).ap()
    wproj = nc.declare_dram_parameter("wproj", [HPC * HD, D], BF16, isOutput=False).ap()
    ones_in = nc.declare_dram_parameter(
        "ones", [(N // P) * (H // 2)], BF16, isOutput=False
    ).ap()
    out = nc.declare_dram_parameter("out", [N, D], F32, isOutput=True).ap()

    DO = D // P          # 6 d-chunks of 128
    NB = N // 512        # 4 n-blocks of 512
    MC = N // P          # 16 m-chunks of 128
    QKC = (2 * HPC * HD) // P   # 6 column blocks of q|k

    with tile.TileContext(nc) as tc, ExitStack() as ctx:
        persist = ctx.enter_context(tc.tile_pool(name="persist", bufs=1))
        outcp = ctx.enter_context(tc.tile_pool(name="outcp", bufs=3))
        small = ctx.enter_context(tc.tile_pool(name="small", bufs=4))
        dramp = ctx.enter_context(tc.tile_pool(name="dramp", bufs=2, space="DRAM"))
        psum_mm = ctx.enter_context(tc.tile_pool(name="psum_mm", bufs=3, space="PSUM"))
        psum_s = ctx.enter_context(tc.tile_pool(name="psum_s", bufs=2, space="PSUM"))
        psum_o = ctx.enter_context(tc.tile_pool(name="psum_o", bufs=1, space="PSUM"))

        # Q^T planes keep the head-pair packing (head h on partitions
        # (h%2)*64..); K^T gets one zero-padded plane per head so the S^T
        # matmul contracts over the full 128 partitions (a half-active PE
        # array is clock-throttled by the HAM to 1.2 GHz).
        qT_sb = persist.tile([P, HPC * HD // P, N], BF16)        # [128, 3, 2048]
        kTz_sb = persist.tile([P, HPC, N], BF16)                 # [128, 6, 2048]
        # V tile: per head 64 V columns + ones column; padded free dim so the
        # O^T matmul lhsT can be widened to M=128 (full-array, HAM-warm)
        v_sb = persist.tile([P, MC, HPC * VW + HD - 1], BF16)    # [128, 16, 453]
        oT_sb = persist.tile([P, HPC * HD // P, N], BF16)        # [128, 3, 2048]
        wp_sb = persist.tile([P, HPC * HD // P, D], BF16)        # [128, 3, 768]

        for p3 in range(HPC * HD // P):
            nc.sync.dma_start(out=wp_sb[:, p3, :], in_=wproj[p3 * P:(p3 + 1) * P, :])
        nc.vector.memset(kTz_sb[:, :, :], 0.0)
        # ones columns for the softmax-denominator rows, DMA'd from a tiny
        # host-provided tensor with a partition-step-0 broadcast read
        for h in range(HPC):
            nc.sync.dma_start(
                out=v_sb[:, :, h * VW + HD],
                in_=bass.AP(
                    tensor=ones_in.tensor,
                    offset=ones_in.offset,
                    ap=[[0, P], [1, MC]],
                ),
            )

        with ExitStack() as early_ctx:
            early = early_ctx.enter_context(tc.tile_pool(name="early", bufs=1))
            xT_sb = early.tile([P, DO, N], BF16)                  # [128, 6, 2048]
            wqkv_sb = early.tile([P, DO, 3 * HPC * HD], BF16)     # [128, 6, 1152]
            for o in range(DO):
                nc.sync.dma_start(out=xT_sb[:, o, :], in_=xT[o * P:(o + 1) * P, :])
                nc.sync.dma_start(
                    out=wqkv_sb[:, o, :], in_=wqkv[o * P:(o + 1) * P, :]
                )

            # ---- Q^T / K^T projection: qkT = (wqkv_qk).T @ x.T ----
            for cb in range(QKC):
                for nb in range(NB):
                    ps = psum_mm.tile([P, 512], F32)
                    for o in range(DO):
                        nc.tensor.matmul(
                            ps[:, :],
                            lhsT=wqkv_sb[:, o, cb * P:(cb + 1) * P],
                            rhs=xT_sb[:, o, nb * 512:(nb + 1) * 512],
                            start=(o == 0),
                            stop=(o == DO - 1),
                        )
                    if cb < QKC // 2:
                        nc.vector.tensor_copy(
                            qT_sb[:, cb, nb * 512:(nb + 1) * 512], ps[:, :]
                        )
                    else:
                        kcb = cb - QKC // 2
                        nc.vector.tensor_copy(
                            kTz_sb[0:HD, 2 * kcb, nb * 512:(nb + 1) * 512],
                            ps[0:HD, :],
                        )
                        nc.vector.tensor_copy(
                            kTz_sb[HD:P, 2 * kcb + 1, nb * 512:(nb + 1) * 512],
                            ps[HD:P, :],
                        )

            # ---- V projection: v = x @ wqkv_v (row chunks) ----
            vcol0 = 2 * HPC * HD
            for mc in range(MC):
                ps = psum_mm.tile([P, 512], F32)
                for o in range(DO):
                    nc.tensor.matmul(
                        ps[:, : HPC * HD],
                        lhsT=xT_sb[:, o, mc * P:(mc + 1) * P],
                        rhs=wqkv_sb[:, o, vcol0: vcol0 + HPC * HD],
                        start=(o == 0),
                        stop=(o == DO - 1),
                    )
                nc.vector.tensor_copy(
                    v_sb[:, :, : HPC * VW].rearrange("p m (h c) -> p m h c", c=VW)[:, mc, :, 0:HD],
                    ps[:, : HPC * HD].rearrange("p (h c) -> p h c", c=HD),
                )

        # ---- attention (pT pool reuses the freed early space) ----
        with ExitStack() as attn_ctx:
            ptp = attn_ctx.enter_context(tc.tile_pool(name="ptp", bufs=2))
            for h in range(HPC):
                kb = (h % 2) * HD
                qpl = h // 2
                for nb in range(NB):
                    pT = ptp.tile([P, MC, 512], BF16)
                    po = psum_o.tile([P, 512], F32)
                    # interleave S-matmuls/exp with the O accumulation so the
                    # PE stream stays dense while ScalarE works through exps
                    for g in range(MC // 2):
                        ps = psum_s.tile([P, 1024], F32)
                        for j in range(2):
                            mc = 2 * g + j
                            nc.tensor.matmul(
                                ps[:, j * 512:(j + 1) * 512],
                                lhsT=kTz_sb[:, h, mc * P:(mc + 1) * P],
                                rhs=qT_sb[:, qpl, nb * 512:(nb + 1) * 512],
                                start=True,
                                stop=True,
                            )
                        nc.scalar.activation(
                            pT[:, 2 * g:2 * g + 2, :].rearrange("p a b -> p (a b)"),
                            ps[:, :],
                            mybir.ActivationFunctionType.Exp,
                            scale=SCALE,
                        )
                        if g >= 1:
                            for mc in (2 * g - 2, 2 * g - 1):
                                nc.tensor.matmul(
                                    po[:, :],
                                    lhsT=v_sb[:, mc, h * VW:h * VW + P],
                                    rhs=pT[:, mc, :],
                                    start=(mc == 0),
                                    stop=False,
                                )
                    for mc in (MC - 2, MC - 1):
                        nc.tensor.matmul(
                            po[:, :],
                            lhsT=v_sb[:, mc, h * VW:h * VW + P],
                            rhs=pT[:, mc, :],
                            start=False,
                            stop=(mc == MC - 1),
                        )
                    # move the accumulated [65,512] out of PSUM promptly,
                    # then normalize: broadcast the denominator row via a
                    # DRAM bounce and apply a fast reciprocal on 64 lanes
                    oTu = small.tile([VW, 512], F32)
                    nc.vector.tensor_copy(oTu[:, :], po[:VW, :])
                    # denominators: 512 values on ONE partition. Spread them
                    # over 64 partitions via a DRAM bounce so the exact DVE
                    # reciprocal runs on 64 lanes, then broadcast back.
                    dsums = dramp.tile([1, 512], F32)
                    nc.gpsimd.dma_start(out=dsums[:, :], in_=oTu[HD:VW, :])
                    spread = small.tile([HD, 8], F32)
                    nc.gpsimd.dma_start(
                        out=spread[:, :],
                        in_=bass.AP(
                            tensor=dsums.tensor,
                            offset=dsums.offset,
                            ap=[[8, HD], [1, 8]],
                        ),
                    )
                    rspread = small.tile([HD, 8], F32)
                    nc.vector.reciprocal(rspread[:, :], spread[:, :])
                    drcp = dramp.tile([1, 512], F32)
                    nc.gpsimd.dma_start(
                        out=bass.AP(
                            tensor=drcp.tensor,
                            offset=drcp.offset,
                            ap=[[8, HD], [1, 8]],
                        ),
                        in_=rspread[:, :],
                    )
                    rcp = small.tile([HD, 512], F32)
                    nc.gpsimd.dma_start(
                        out=rcp[:, :],
                        in_=bass.AP(
                            tensor=drcp.tensor,
                            offset=drcp.offset,
                            ap=[[0, HD], [1, 512]],
                        ),
                    )
                    nc.vector.tensor_mul(
                        oT_sb[kb:kb + HD, qpl, nb * 512:(nb + 1) * 512],
                        oTu[0:HD, :],
                        rcp[:, :],
                    )

            # ---- output projection: out = oT.T @ wproj ----
            PH = HPC * HD // P  # 3
            for mc in range(MC):
                for half in range(2):
                    ps = psum_mm.tile([P, 512], F32)
                    for p3 in range(PH):
                        nc.tensor.matmul(
                            ps[:, : D // 2],
                            lhsT=oT_sb[:, p3, mc * P:(mc + 1) * P],
                            rhs=wp_sb[:, p3, half * (D // 2):(half + 1) * (D // 2)],
                            start=(p3 == 0),
                            stop=(p3 == PH - 1),
                        )
                    oc = outcp.tile([P, D // 2], F32)
                    nc.vector.tensor_copy(oc[:, :], ps[:, : D // 2])
                    nc.sync.dma_start(
                        out=out[mc * P:(mc + 1) * P, half * (D // 2):(half + 1) * (D // 2)],
                        in_=oc[:, :],
                    )

    _split_waits(nc)
    return nc


_NC_CACHE = [None]


def _get_nc():
    if _NC_CACHE[0] is None:
        _NC_CACHE[0] = _build_nc()
    return _NC_CACHE[0]


def _make_in_maps(x, W_qkv, W_proj):
    import ml_dtypes

    bf16 = ml_dtypes.bfloat16
    in_maps = []
    for c in range(NCORES):
        b = c // 2
        h0 = (c % 2) * HPC
        qcols = W_qkv[:, h0 * HD:(h0 + HPC) * HD]
        kcols = W_qkv[:, D + h0 * HD: D + (h0 + HPC) * HD]
        vcols = W_qkv[:, 2 * D + h0 * HD: 2 * D + (h0 + HPC) * HD]
        in_maps.append(
            {
                "xT": np.ascontiguousarray(x[b].T).astype(bf16),
                "wqkv": np.concatenate([qcols, kcols, vcols], axis=1).astype(bf16),
                "wproj": np.ascontiguousarray(
                    W_proj[h0 * HD:(h0 + HPC) * HD, :]
                ).astype(bf16),
                "ones": np.ones((N // P) * (H // 2), dtype=bf16),
            }
        )
    return in_maps


def _run(inputs, trace=False):
    x = np.asarray(inputs["x"], dtype=np.float32)
    W_qkv = np.asarray(inputs["W_qkv"], dtype=np.float32)
    W_proj = np.asarray(inputs["W_proj"], dtype=np.float32)
    b_proj = np.asarray(inputs["b_proj"], dtype=np.float32)

    if trace:
        _install_ntff_shim()
    nc = _get_nc()
    res = run_bass_kernel_spmd(
        nc, _make_in_maps(x, W_qkv, W_proj), core_ids=list(range(NCORES)),
        trace=trace,
    )
    parts = res.results
    out = np.empty((B, N, D), dtype=np.float32)
    for b in range(B):
        out[b] = parts[2 * b]["out"] + parts[2 * b + 1]["out"] + b_proj
    return out, res


def kernel(**inputs) -> np.ndarray:
    out, _ = _run(inputs, trace=False)
    return out


def run_traced(inputs):
    return _run(inputs, trace=True)
